# revision 1
# baseline (speedup 1.0000x reference)
"""Multi-head attention (B=2, S=2048, D=1024, H=16) on 8 Trainium2 cores.

Sharding: core c handles batch b = c//4 and head group g = c%4 (4 heads).
Output projection is row-sharded over head dims; per-core partial outputs
are summed on the host (bias folded into the g==0 cores).

Per-core kernel (all matmul operands bf16, fp32 accumulation):
  S^T[j,i] = K_h^T(stationary) x Q_h^T(moving)  (d=64 contraction; two heads
             packed into PE row-groups 0-1 / 2-3 so the pair runs concurrently)
  E = exp(SCALE * S^T)  on ACT, straight out of PSUM, bf16 into SBUF
  O_aug[0:65,i] = sum_j V_aug[j, 0:65]^T E[j, i]   (V augmented with a ones
             column FIRST => row 0 of O_aug is the softmax denominator, which
             physically lives on partition 0 so gpsimd partition_broadcast
             works on hardware)
  A^T[hd, i] = O_aug[1:65] * (1/O_aug[0])   (fast reciprocal + broadcast),
             placed into the projection operand tile via SBUF->SBUF DMA
  y[i, mo] += A^T-chunk(stationary) x W^T(moving)  + bias

Pipeline: per block (i-super x head-pair), PV of the previous block runs as
dense PE bursts before/between the QK ping-pong so the PE stays HAM-warm;
a dummy 16-matmul warmup flips HAM to 8/8 at kernel start.
"""

import sys

sys.path.insert(0, "/opt/trn_rl_repo")

from contextlib import ExitStack

import numpy as np
import ml_dtypes

import concourse.bass as bass
import concourse.tile as tile
from concourse import bacc, mybir

N_CORES = 8
B, S, D_MODEL = 2, 2048, 1024
NUM_HEADS, D_K = 16, 64
H_PER_CORE = 4            # heads per core, as 2 pairs of 2
SCALE = D_K ** -0.5
IS = 1024                 # i-super width (2 supers cover S)
N_IS = S // IS            # 2
JB = S // 128             # 16 j-blocks
VA_W = 128                # ones col 0, zeros 1-63, v at 64-127 (64-aligned)
ET_BUFS = 48

F32 = mybir.dt.float32
BF16 = mybir.dt.bfloat16
AF = mybir.ActivationFunctionType
# (pair, i0, iw): last pair's second i-super split in two to shorten the tail
BLOCKS = [(0, 0, 1024), (1, 0, 1024), (0, 1024, 1024),
          (1, 1024, 512), (1, 1536, 256), (1, 1792, 256)]


def ds(start, size):
    return slice(start, start + size)


def _trace(ctx: ExitStack, tc: tile.TileContext, io: dict):
    nc = tc.nc

    const = ctx.enter_context(tc.tile_pool(name="const", bufs=1))
    etp = ctx.enter_context(tc.tile_pool(name="et", bufs=ET_BUFS))
    normp = ctx.enter_context(tc.tile_pool(name="norm", bufs=2))
    atp = ctx.enter_context(tc.tile_pool(name="at", bufs=1))
    youtp = ctx.enter_context(tc.tile_pool(name="yout", bufs=2))
    psS = ctx.enter_context(tc.tile_pool(name="psS", bufs=2, space="PSUM"))
    psO = ctx.enter_context(tc.tile_pool(name="psO", bufs=1, space="PSUM"))
    psY = ctx.enter_context(tc.tile_pool(name="psY", bufs=2, space="PSUM"))

    # ---- resident inputs ----
    # first 512-col slivers of kt0/qt0 land first so the warmup and block-0
    # QK can start while the bulk of the inputs is still in flight
    kt_sb = [const.tile([128, S], BF16, tag=f"kt{p}", name=f"kt{p}")
             for p in range(2)]
    qt_sb = [const.tile([128, S], BF16, tag=f"qt{p}", name=f"qt{p}")
             for p in range(2)]
    wt_sb = []
    nc.sync.dma_start(kt_sb[0][:, 0:512], io["kt"][0][:, 0:512])
    nc.sync.dma_start(qt_sb[0][:, 0:512], io["qt"][0][:, 0:512])
    nc.sync.dma_start(kt_sb[0][:, 512:S], io["kt"][0][:, 512:S])
    nc.sync.dma_start(qt_sb[0][:, 512:S], io["qt"][0][:, 512:S])
    nc.sync.dma_start(kt_sb[1][:], io["kt"][1])
    nc.sync.dma_start(qt_sb[1][:], io["qt"][1])
    va_sb = const.tile([128, JB * H_PER_CORE * VA_W], BF16, tag="va")
    nc.sync.dma_start(va_sb[:], io["va"][:])
    for p in range(2):
        t = const.tile([128, D_MODEL], BF16, tag=f"wt{p}")
        nc.sync.dma_start(t[:], io["wt"][p])
        wt_sb.append(t)
    at_sb = [atp.tile([128, S], BF16, tag=f"at{p}", name=f"at{p}")
             for p in range(2)]

    # ---- HAM warmup: ~16 dense matmuls flip the PE clock to 8/8 early ----
    wps = psY.tile([128, 512], F32, tag="Y", name="warm_ps")
    for _ in range(10):
        nc.tensor.matmul(wps[:], kt_sb[0][:, 0:128], kt_sb[0][:, 0:512],
                         start=True, stop=True, skip_group_check=True)
    wsb = youtp.tile([1, 512], F32, tag="warm_out", name="warm_out")
    nc.vector.tensor_copy(wsb[:], wps[0:1, :])
    nc.sync.dma_start(io["warm"][:], wsb[:])

    ET = {}    # (block_idx, h2) -> list of 16 E tiles
    PSO = {}   # (block_idx, h2) -> psum tile

    def emit_qk_exp(t, jb):
        pr, i0, iw = BLOCKS[t]
        cw = min(512, iw)
        # interleave the two heads' matmuls (A0,B0,A1,B1): each LDWEIGHTS
        # then overlaps the other head's in-flight K=64 matmul (disjoint
        # row-groups), instead of serializing behind its own head's matmul
        sps = [psS.tile([128, iw], F32, tag="S", name="sp") for _ in range(2)]
        for nch in range(iw // cw):
            for h2 in range(2):
                nc.tensor.matmul(
                    sps[h2][:, ds(nch * cw, cw)],
                    kt_sb[pr][ds(h2 * 64, 64), ds(jb * 128, 128)],
                    qt_sb[pr][ds(h2 * 64, 64), ds(i0 + nch * cw, cw)],
                    start=True, stop=True,
                )
        for h2 in range(2):
            e = etp.tile([128, iw], BF16, tag="et", name="e")
            nc.scalar.activation(e[:], sps[h2][:], AF.Exp, scale=SCALE)
            ET[(t, h2)][jb] = e

    def emit_pv(t, h2, jbps):
        pr, i0, iw = BLOCKS[t]
        h = pr * 2 + h2
        if (t, h2) not in PSO:
            PSO[(t, h2)] = psO.tile([128, iw], F32, tag="O", name="psO")
        O = PSO[(t, h2)]
        cw = min(512, iw)
        for jbp in jbps:
            for nch in range(iw // cw):
                nc.tensor.matmul(
                    O[0:128, ds(nch * cw, cw)],
                    va_sb[:, ds(jbp * H_PER_CORE * VA_W + h * VA_W, VA_W)],
                    ET[(t, h2)][jbp][:, ds(nch * cw, cw)],
                    start=(jbp == 0), stop=(jbp == JB - 1),
                    skip_group_check=True,
                )

    def emit_norm(t, h2):
        pr, i0, iw = BLOCKS[t]
        O = PSO[(t, h2)]
        rr = normp.tile([1, iw], F32, tag="rr", name="rr")
        nc.vector.reciprocal_approx_fast(rr[:], O[0:1, :])
        bc = normp.tile([128, iw], F32, tag="bc", name="bc")
        nc.gpsimd.partition_broadcast(bc[:], rr[0:1, :])
        nm = normp.tile([128, iw], BF16, tag="nm", name="nm")
        nc.vector.tensor_mul(nm[64:128, :], O[64:128, :], bc[64:128, :])
        nc.sync.dma_start(at_sb[pr][ds(h2 * 64, 64), ds(i0, iw)],
                          nm[64:128, :])
        del ET[(t, h2)]

    def emit_proj(unit, eng="vector"):
        ic, moch = unit
        Y = psY.tile([128, 512], F32, tag="Y")
        for hd2 in range(2):
            nc.tensor.matmul(
                Y[:],
                at_sb[hd2][:, ds(ic * 128, 128)],
                wt_sb[hd2][:, ds(moch * 512, 512)],
                start=(hd2 == 0), stop=(hd2 == 1),
                skip_group_check=True,
            )
        ysb = youtp.tile([128, 512], F32, tag="y")
        if eng == "vector":
            nc.vector.tensor_copy(ysb[:], Y[:])
        else:
            nc.scalar.copy(ysb[:], Y[:])
        nc.sync.dma_start(io["y"][ds(ic * 128, 128), ds(moch * 512, 512)],
                          ysb[:])

    # proj unit (ic, m) is ready once every block covering i-chunk ic (for
    # both pairs) has been normalized; with the block order above that is:
    #   ic 0-3  after block 2 step 7   -> run in block 2 steps 8-15
    #   ic 4-7  after block 3 step 7   -> run in block 3 steps 8-15
    #   ic 8-11 after block 4 step 7   -> run in block 4 steps 8-15
    #   ic 12-15 in the tail
    proj_sched = {
        2: [(ic, m) for ic in range(0, 4) for m in range(2)],
        3: [(ic, m) for ic in range(4, 8) for m in range(2)],
        4: [(ic, m) for ic in range(8, 12) for m in range(2)],
        5: [(ic, m) for ic in range(12, 14) for m in range(2)],
    }
    for t in range(len(BLOCKS)):
        for h2 in range(2):
            ET[(t, h2)] = [None] * JB
        projq = list(proj_sched.get(t, []))
        for jb in range(JB):
            emit_qk_exp(t, jb)
            if t > 0 and jb < 8:
                # PV of previous block: head 1 during steps 0-7 (head 0 was
                # already folded into the previous block's steps 8-15)
                emit_pv(t - 1, 1, [2 * jb, 2 * jb + 1])
                if jb == 7:
                    emit_norm(t - 1, 1)
            if jb >= 8:
                # fold this block's own head-0 PV into steps 8-15; its E
                # tiles jbp<=2*(jb-8)+1 are already drained by ACT by now
                emit_pv(t, 0, [2 * (jb - 8), 2 * (jb - 8) + 1])
                if jb == JB - 1:
                    emit_norm(t, 0)
            if projq and jb >= 8:
                emit_proj(projq.pop(0))
        for u in projq:
            emit_proj(u)

    # tail: head-1 PV of the last block, then the final projection units
    last = len(BLOCKS) - 1
    emit_pv(last, 1, list(range(JB)))
    emit_norm(last, 1)
    for ic in range(14, 16):
        for m in range(2):
            emit_proj((ic, m), eng="scalar")


_CACHED_NC = None


def _build():
    global _CACHED_NC
    if _CACHED_NC is not None:
        return _CACHED_NC
    nc = bacc.Bacc("TRN2", target_bir_lowering=False, debug=False,
                   num_devices=N_CORES)
    io = {
        "qt": nc.dram_tensor("qt", [2, 128, S], BF16, kind="ExternalInput").ap(),
        "kt": nc.dram_tensor("kt", [2, 128, S], BF16, kind="ExternalInput").ap(),
        "va": nc.dram_tensor("va", [128, JB * H_PER_CORE * VA_W], BF16,
                             kind="ExternalInput").ap(),
        "wt": nc.dram_tensor("wt", [2, 128, D_MODEL], BF16,
                             kind="ExternalInput").ap(),
        "y": nc.dram_tensor("y", [S, D_MODEL], F32, kind="ExternalOutput").ap(),
        "warm": nc.dram_tensor("warm", [1, 512], F32,
                               kind="ExternalOutput").ap(),
    }
    with tile.TileContext(nc) as tc:
        with ExitStack() as ctx:
            _trace(ctx, tc, io)
    nc.compile()
    _CACHED_NC = nc
    return nc


def _core_inputs(q, k, v, W, b, core):
    bb, g = divmod(core, 4)
    hd0 = g * H_PER_CORE * D_K  # 256 per group
    ncol = H_PER_CORE * D_K
    bf = ml_dtypes.bfloat16

    qt = np.ascontiguousarray(q[bb, :, hd0:hd0 + ncol].T).reshape(2, 128, S)
    kt = np.ascontiguousarray(k[bb, :, hd0:hd0 + ncol].T).reshape(2, 128, S)
    v_sl = v[bb, :, hd0:hd0 + ncol].reshape(S, H_PER_CORE, D_K)
    va = np.concatenate(
        [np.ones((S, H_PER_CORE, 1), np.float32),
         np.zeros((S, H_PER_CORE, 63), np.float32), v_sl], axis=2
    ).reshape(JB, 128, H_PER_CORE * VA_W).transpose(1, 0, 2).reshape(
        128, JB * H_PER_CORE * VA_W)
    wt = np.ascontiguousarray(W[:, hd0:hd0 + ncol].T).reshape(2, 128, D_MODEL)
    return {
        "qt": qt.astype(bf),
        "kt": kt.astype(bf),
        "va": np.ascontiguousarray(va).astype(bf),
        "wt": wt.astype(bf),
    }


def run(inputs, trace=False, trace_kwargs=None):
    from concourse.bass_utils import run_bass_kernel_spmd

    q = np.asarray(inputs["q"], np.float32)
    k = np.asarray(inputs["k"], np.float32)
    v = np.asarray(inputs["v"], np.float32)
    W = np.asarray(inputs["W"], np.float32)
    b = np.asarray(inputs["b"], np.float32)

    nc = _build()
    in_maps = [_core_inputs(q, k, v, W, b, c) for c in range(N_CORES)]
    res = run_bass_kernel_spmd(nc, in_maps, core_ids=list(range(N_CORES)),
                               trace=trace, **(trace_kwargs or {}))
    out = np.empty((B, S, D_MODEL), np.float32)
    for bb in range(B):
        acc = res.results[bb * 4 + 0]["y"].astype(np.float32)
        for g in range(1, 4):
            acc = acc + res.results[bb * 4 + g]["y"]
        out[bb] = acc + b[None, :]
    return out, res


def kernel(**inputs):
    out, _ = run(inputs)
    return out

